# revision 1
# baseline (speedup 1.0000x reference)
"""Balanced CE loss + accuracy on 8 Trainium2 NeuronCores (Bass/Tile).

Reference computation (N = 16777216 elements):
    loss = -sum(where(t==1, 1.6*log(p), 0.4*log(1-p))) / N
    acc  = mean(round(p) == t)

Strategy (data-parallel over N, no collectives needed):
  Shard N across 8 cores; per core stream 2048-column sub-chunks
  ([128, 2048] tiles, ~2 MB DMA each) so DMA, DVE and ACT stay smoothly
  pipelined. The identity log(1)=0 avoids an elementwise select:
    y1 = p if t==1 else 1      -> sum(ln(y1)) = sum_{t==1} ln(p)     =: A1
    y0 = 1-p if t==0 else 1    -> sum(ln(y0)) = sum_{t==0} ln(1-p)   =: B0
  with a1 = (p-1)*t = y1-1 and a0 = (t-1)*p = y0-1, each one fused DVE
  scalar_tensor_tensor op (int32 t converted on read).  ACT computes
  Ln(a+1) in-place per round (a round = 1-2 sub-chunks) with fused
  free-dim accumulation.  Accuracy = C1 + C0 - N from threshold counts:
    C1 = #(y1 >= 0.5): DVE is_ge at 2x into a bf16 mask, partition-
         reduced by idle TensorE matmuls (ones^T @ mask) into PSUM.
    C0 = #(y0 >= 0.5): ACT Sign pass on ln(y0) (sign(ln(y0)+ln2)) for
         the first rounds; DVE is_ge mask for the last round so the
         kernel tail is not ACT-bound.
  Per-(partition, round) partials are DMA'd out; host reduces in f64.
"""

import sys

if "/opt/trn_rl_repo" not in sys.path:
    sys.path.insert(0, "/opt/trn_rl_repo")

import numpy as np

import concourse.bass as bass
import concourse.bacc as bacc
import concourse.tile as tile
from concourse import mybir
from concourse.bass_utils import run_bass_kernel_spmd

N_CORES = 8
N = 16777216
P = 128
SHARD = N // N_CORES          # 2097152 elements per core
COLS = SHARD // P             # 16384 columns per core
SUB = 2048                    # DVE/DMA sub-chunk columns
NSUB = COLS // SUB            # 8 sub-chunks
# ACT rounds in units of sub-chunks; last round's C0 handled on DVE.
ROUNDS = [1, 1, 2, 2, 1, 1]
assert sum(ROUNDS) == NSUB
NR = len(ROUNDS)
SIGN_SUBS = 5                 # sub-chunks [0, SIGN_SUBS) use ACT Sign for C0
MMCOL = 512                   # matmul free-dim tile (one PSUM bank)

AF = mybir.ActivationFunctionType
OP = mybir.AluOpType
LN2 = 0.6931471805599453

# columns covered by the ACT-Sign C0 path (per core)
SIGN_COLS = SIGN_SUBS * SUB

_NC_CACHE = None


def build_bass():
    """Build the single-core Bass program (SPMD across 8 cores)."""
    global _NC_CACHE
    if _NC_CACHE is not None:
        return _NC_CACHE

    nc = bacc.Bacc("TRN2", target_bir_lowering=False, debug=False)

    p_in = nc.dram_tensor("p_in", [SHARD], mybir.dt.float32, kind="ExternalInput").ap()
    t_in = nc.dram_tensor("t_in", [SHARD], mybir.dt.int32, kind="ExternalInput").ap()
    # acc_act columns: [r] sum ln(y1), [NR+r] sum ln(y0) per round r;
    # [2NR+s] sum sign(y0-0.5) per sign sub-chunk s (unused cols stay 0)
    acc_act = nc.dram_tensor("acc_act", [P, 2 * NR + NSUB], mybir.dt.float32, kind="ExternalOutput").ap()
    # acc_dve columns: [0] C1 partial, [1] C0 partial (last round)
    acc_dve = nc.dram_tensor("acc_dve", [P, 2], mybir.dt.float32, kind="ExternalOutput").ap()

    n_mm1 = COLS // MMCOL                          # cnt1 matmuls
    n_mm0 = (COLS - SIGN_COLS) // MMCOL            # cnt0 matmuls (last round)

    with tile.TileContext(nc) as tc:
        with (
            tc.tile_pool(name="io", bufs=4) as io_pool,
            tc.tile_pool(name="work", bufs=2) as work_pool,
            tc.tile_pool(name="cmp", bufs=3) as cmp_pool,
            tc.tile_pool(name="psum", bufs=1, space=bass.MemorySpace.PSUM) as psum_pool,
            tc.tile_pool(name="misc", bufs=1) as misc_pool,
        ):
            ones = misc_pool.tile([P, P], mybir.dt.bfloat16, tag="ones")
            nc.gpsimd.memset(ones[:], 1.0)
            # ln(2) Sign-bias as a tracked tile (avoids a pre-context
            # const memset + all-engine barrier that delays the first DMA)
            ln2c = misc_pool.tile([P, 1], mybir.dt.float32, tag="ln2c")
            nc.gpsimd.memset(ln2c[:], LN2)
            junk = misc_pool.tile([P, MMCOL], mybir.dt.float32, tag="junk")
            acc_act_sb = misc_pool.tile([P, 2 * NR + NSUB], mybir.dt.float32, tag="acca")
            acc_dve_sb = misc_pool.tile([P, 2], mybir.dt.float32, tag="accd")
            nc.gpsimd.memset(acc_act_sb[:], 0.0)
            ps1 = psum_pool.tile([P, MMCOL], mybir.dt.float32, tag="ps1")
            ps0 = psum_pool.tile([P, MMCOL], mybir.dt.float32, tag="ps0")

            sub = 0          # global sub-chunk index
            mm1 = 0
            mm0 = 0
            for r, nsubs in enumerate(ROUNDS):
                rc = nsubs * SUB
                a1 = work_pool.tile([P, rc], mybir.dt.float32, tag="a1")
                a0 = work_pool.tile([P, rc], mybir.dt.float32, tag="a0")
                for k in range(nsubs):
                    off = sub * SUB * P
                    p_t = io_pool.tile([P, SUB], mybir.dt.float32, tag="p")
                    t_t = io_pool.tile([P, SUB], mybir.dt.int32, tag="t")
                    # split the very first sub-chunk's DMA/compute in half so
                    # the pipeline starts ~2us earlier
                    nhalf = 2 if sub == 0 else 1
                    hc = SUB // nhalf
                    for h in range(nhalf):
                        ho = off + h * hc * P
                        hs = slice(h * hc, (h + 1) * hc)
                        nc.sync.dma_start(
                            p_t[:, hs], p_in[ho : ho + hc * P].rearrange("(p f) -> p f", p=P)
                        )
                        nc.sync.dma_start(
                            t_t[:, hs], t_in[ho : ho + hc * P].rearrange("(p f) -> p f", p=P)
                        )
                        asl = slice(k * SUB + h * hc, k * SUB + (h + 1) * hc)
                        # a1 = (p - 1) * t ;  a0 = (t - 1) * p
                        nc.vector.scalar_tensor_tensor(a1[:, asl], p_t[:, hs], -1.0, t_t[:, hs], OP.add, OP.mult)
                        nc.vector.scalar_tensor_tensor(a0[:, asl], t_t[:, hs], -1.0, p_t[:, hs], OP.add, OP.mult)
                    sl = slice(k * SUB, (k + 1) * SUB)
                    # C1 mask (a1 >= -0.5) at 2x, partition-reduced on TensorE
                    cm = cmp_pool.tile([P, SUB], mybir.dt.bfloat16, tag="cm")
                    nc.vector.tensor_scalar(cm[:], a1[:, sl], -0.5, None, OP.is_ge)
                    for j in range(SUB // MMCOL):
                        nc.tensor.matmul(
                            ps1[:], ones[:], cm[:, j * MMCOL : (j + 1) * MMCOL],
                            start=(mm1 == 0), stop=(mm1 == n_mm1 - 1),
                        )
                        mm1 += 1
                    if sub >= SIGN_SUBS:
                        # tail subs: C0 via DVE mask too (keeps ACT off the tail)
                        cz = cmp_pool.tile([P, SUB], mybir.dt.bfloat16, tag="cz")
                        nc.vector.tensor_scalar(cz[:], a0[:, sl], -0.5, None, OP.is_ge)
                        for j in range(SUB // MMCOL):
                            nc.tensor.matmul(
                                ps0[:], ones[:], cz[:, j * MMCOL : (j + 1) * MMCOL],
                                start=(mm0 == 0), stop=(mm0 == n_mm0 - 1),
                            )
                            mm0 += 1
                    sub += 1

                # ACT ops write in-place; accumulate per-partition round sums
                nc.scalar.activation(a1[:], a1[:], AF.Ln, bias=1.0,
                                     accum_out=acc_act_sb[:, r : r + 1])
                nc.scalar.activation(a0[:], a0[:], AF.Ln, bias=1.0,
                                     accum_out=acc_act_sb[:, NR + r : NR + r + 1])
                for k in range(nsubs):
                    s_g = sub - nsubs + k     # global sub index of slice k
                    if s_g < SIGN_SUBS:
                        ksl = slice(k * SUB, (k + 1) * SUB)
                        nc.scalar.activation(a0[:, ksl], a0[:, ksl], AF.Sign, bias=ln2c[:, 0:1],
                                             accum_out=acc_act_sb[:, 2 * NR + s_g : 2 * NR + s_g + 1])

            # fold the PSUM count matrices (128 identical rows) into columns
            nc.vector.tensor_scalar(junk[:], ps1[:], 1.0 / P, None, OP.mult,
                                    OP.add, accum_out=acc_dve_sb[:, 0:1])
            nc.vector.tensor_scalar(junk[:], ps0[:], 1.0 / P, None, OP.mult,
                                    OP.add, accum_out=acc_dve_sb[:, 1:2])

            nc.sync.dma_start(acc_act[:], acc_act_sb[:])
            nc.sync.dma_start(acc_dve[:], acc_dve_sb[:])

    nc.finalize()
    _NC_CACHE = nc
    return nc


def make_in_maps(input, target):
    inp = np.ascontiguousarray(np.asarray(input, dtype=np.float32)).reshape(
        N_CORES, SHARD
    )
    tgt = np.ascontiguousarray(np.asarray(target, dtype=np.int32)).reshape(
        N_CORES, SHARD
    )
    return [{"p_in": inp[c], "t_in": tgt[c]} for c in range(N_CORES)]


def combine(results):
    """Host-side unshard: reduce the 8 cores' partial sums -> (loss, acc)."""
    A1 = B0 = S0 = C1 = C0m = 0.0
    for r in results:
        aa = np.asarray(r["acc_act"], dtype=np.float64)
        ad = np.asarray(r["acc_dve"], dtype=np.float64)
        A1 += aa[:, 0:NR].sum()
        B0 += aa[:, NR : 2 * NR].sum()
        S0 += aa[:, 2 * NR : 2 * NR + NSUB].sum()
        C1 += ad[:, 0].sum()
        C0m += ad[:, 1].sum()
    loss = -(1.6 * A1 + 0.4 * B0) / N
    n_sign = N_CORES * SIGN_COLS * P          # elements counted via Sign
    C0 = (S0 + n_sign) / 2.0 + C0m
    acc = (C1 + C0 - N) / N
    return np.float32(loss), np.float32(acc)


def run_on_hw(input, target, **spmd_kwargs):
    nc = build_bass()
    in_maps = make_in_maps(input, target)
    return run_bass_kernel_spmd(nc, in_maps, list(range(N_CORES)), **spmd_kwargs)


def kernel(input, target):
    br = run_on_hw(input, target)
    return combine(br.results)

